# revision 18
# baseline (speedup 1.0000x reference)
"""Cosine-similarity KNN (top-10 of 1M docs x 256 dims) on 8 Trainium2 cores.

Strategy (memory-bound problem):
  - Shard the docs table row-wise: 125,000 docs per core.
  - Each core streams its shard HBM->SBUF in 2 MB chunks (16 docs per
    partition per chunk, 16 KB contiguous per partition per DMA) and computes
    the raw dot product <query, doc> for every doc with one fused DVE
    scalar_tensor_tensor (multiply + row-sum accumulator) per 128-doc tile.
  - Ranking by raw dot is used only for candidate *selection* (l2(query) is a
    constant, and doc norms concentrate tightly around sqrt(256)), with a huge
    margin: each core keeps the top-8 dots per partition (1024 candidates per
    core, ~100x more than needed) via the DVE Max8/MaxIndex instructions.
  - The host gathers 8 x 1024 candidate doc ids, recomputes the exact fp32
    cosine for those ~8K rows, and reduces to the global top-10 (values and
    int32 indices), matching the reference numerics.
"""

import sys

for _p in ("/opt/trn_rl_repo",):
    if _p not in sys.path:
        sys.path.insert(0, _p)

import numpy as np

import concourse.bacc as bacc
import concourse.mybir as mybir
from concourse import tile
from concourse.bass_utils import run_bass_kernel_spmd

EPS = 1e-12
TOP_K = 10
D = 256
N_CORES = 8
G = 16                      # docs per partition per chunk
P = 128                     # partitions
CHUNK = P * G               # 2048 docs per chunk

F32 = mybir.dt.float32
U32 = mybir.dt.uint32

_NC_CACHE = {}
LAST_RESULT = None          # BassKernelResults of the last hardware run


def _build_nc(
    shard: int,
    chunks_override: int | None = None,
    mode: str = "full",
    bf16: bool = False,
):
    """Build the single-core Bass program for a shard of `shard` docs.

    chunks_override / mode ("full" | "dma_only" | "compute_only"): timing-only
    variants over the same-shaped input (results are then meaningless).
    bf16: docs/query tiles in bf16 (SWDGE cast during DMA); dots stay fp32."""
    chunks = shard // CHUNK
    tail = shard % CHUNK
    if chunks_override is not None:
        chunks, tail = chunks_override, 0
    n_cols = chunks * G + (G if tail else 0)
    DT = mybir.dt.bfloat16 if bf16 else F32
    assert n_cols >= 8

    nc = bacc.Bacc(None, target_bir_lowering=False, debug=False)

    q_ext = nc.declare_dram_parameter("query", [1, D], F32, isOutput=False)
    docs_ext = nc.declare_dram_parameter("docs", [shard, D], F32, isOutput=False)
    vals_ext = nc.declare_dram_parameter("vals8", [P, 8], F32, isOutput=True)
    idx_ext = nc.declare_dram_parameter("idx8", [P, 8], U32, isOutput=True)

    with tile.TileContext(nc) as tc:
        with (
            tc.tile_pool(name="persist", bufs=1) as persist,
            tc.tile_pool(name="stream", bufs=4) as stream,
        ):
            qb = persist.tile([P, D], DT)
            if bf16:
                nc.gpsimd.dma_start(
                    out=qb[:, :], in_=q_ext[:, :].to_broadcast((P, D))
                )
            else:
                nc.sync.dma_start(
                    out=qb[:, :], in_=q_ext[:, :].to_broadcast((P, D))
                )

            dots = persist.tile([P, n_cols], F32)

            def load_chunk(buf, r0):
                src = docs_ext[r0 : r0 + CHUNK, :].rearrange(
                    "(p g) d -> p (g d)", p=P
                )
                if bf16:
                    nc.gpsimd.dma_start(out=buf[:, :], in_=src)  # casts f32->bf16
                else:
                    nc.sync.dma_start(out=buf[:, :], in_=src)

            def do_tile(buf, t, col):
                # dot[p, col] = sum_d buf[p, t*D+d] * q[d]
                # (scalar_tensor_tensor: out = (in0 op0 scalar) op1 in1,
                #  accum_out = sum(out); tensor_tensor_reduce crashes the
                #  device on this runtime, this opcode is the working one.)
                sl = buf[:, t * D : (t + 1) * D]
                nc.vector.scalar_tensor_tensor(
                    out=sl,
                    in0=sl,
                    scalar=1.0,
                    in1=qb[:, :],
                    op0=mybir.AluOpType.mult,
                    op1=mybir.AluOpType.mult,
                    accum_out=dots[:, col : col + 1],
                )

            if mode != "full":
                nc.vector.memset(dots[:, :], 0.0)
            real_chunks = shard // CHUNK
            buf0 = None
            for c in range(chunks):
                r0 = (c % real_chunks) * CHUNK
                if mode == "compute_only" and buf0 is not None:
                    buf = buf0
                else:
                    buf = stream.tile([P, G * D], DT, tag="docs")
                    load_chunk(buf, r0)
                    buf0 = buf
                if mode != "dma_only":
                    for t in range(G):
                        do_tile(buf, t, c * G + t)

            if tail:
                # Tail: one more FULL chunk that overlaps the previous one
                # (docs [shard-CHUNK, shard)). The overlap produces duplicate
                # scores; the host dedupes by doc id. No pad handling needed.
                assert shard >= CHUNK
                bufT = stream.tile([P, G * D], DT, tag="docs")
                load_chunk(bufT, shard - CHUNK)
                for t in range(G):
                    do_tile(bufT, t, chunks * G + t)

            vals8 = persist.tile([P, 8], F32)
            idx8 = persist.tile([P, 8], U32)
            nc.vector.max(vals8[:, :], dots[:, :])
            nc.vector.max_index(idx8[:, :], vals8[:, :], dots[:, :])
            nc.sync.dma_start(out=vals_ext[:, :], in_=vals8[:, :])
            nc.sync.dma_start(out=idx_ext[:, :], in_=idx8[:, :])

    nc.finalize()
    return nc


USE_BF16 = False    # flipped after HW probes validate the cast-DMA/bf16 path


def _get_nc(shard: int, bf16: bool = False):
    key = (shard, bf16)
    if key not in _NC_CACHE:
        _NC_CACHE[key] = _build_nc(shard, bf16=bf16)
    return _NC_CACHE[key]


def _merge_host(query, docs, idx8_per_core, shard):
    """Exact fp32 cosine on the device-selected candidates; global top-10."""
    q = np.asarray(query, dtype=np.float32).reshape(D)
    chunks = shard // CHUNK
    cand = []
    p_col = np.arange(P, dtype=np.int64)[:, None]
    for i, idx8 in enumerate(idx8_per_core):
        j = idx8.astype(np.int64)          # [128, 8] column index into dots
        c, t = j // G, j % G
        r0 = np.where(c < chunks, c * CHUNK, shard - CHUNK)
        doc = i * shard + r0 + p_col * G + t
        cand.append(doc.ravel())
    cand = np.unique(np.concatenate(cand))
    cand = cand[cand < docs.shape[0]]      # paranoia

    d = np.asarray(docs[cand], dtype=np.float32)
    l2q = np.sqrt(np.sum(np.maximum(q * q, EPS), dtype=np.float32).astype(np.float32))
    l2d = np.sqrt(np.sum(np.maximum(d * d, EPS), axis=1, dtype=np.float32))
    dot = (d @ q).astype(np.float32)
    cos = dot / (l2q * l2d)

    order = np.argsort(-cos, kind="stable")[:TOP_K]
    vals = cos[order].astype(np.float32)
    idx = cand[order].astype(np.int32)
    return vals, idx


def _run_sim(nc, in_maps):
    """CoreSim path for functional validation (no hardware)."""
    from concourse import bass_interp

    sim = bass_interp.MultiCoreSim(nc, len(in_maps))
    for i, m in enumerate(in_maps):
        for k, v in m.items():
            sim.cores[i].tensor(k)[:] = v
    sim.simulate()
    return [
        {
            "vals8": np.array(sim.cores[i].mem_tensor("vals8")),
            "idx8": np.array(sim.cores[i].mem_tensor("idx8")),
        }
        for i in range(len(in_maps))
    ]


def _kernel_impl(query, docs, n_cores, use_sim=False, trace=False):
    global LAST_RESULT
    n = docs.shape[0]
    assert n % n_cores == 0
    shard = n // n_cores
    nc = _get_nc(shard, bf16=USE_BF16)

    query = np.ascontiguousarray(np.asarray(query, dtype=np.float32))
    docs = np.asarray(docs, dtype=np.float32)
    in_maps = [
        {"query": query, "docs": docs[i * shard : (i + 1) * shard]}
        for i in range(n_cores)
    ]

    if use_sim:
        results = _run_sim(nc, in_maps)
    else:
        r = run_bass_kernel_spmd(
            nc, in_maps, core_ids=list(range(n_cores)), trace=trace
        )
        LAST_RESULT = r
        results = r.results

    idx8s = [np.asarray(results[i]["idx8"]) for i in range(n_cores)]
    return _merge_host(query, docs, idx8s, shard)


def kernel(query, docs):
    return _kernel_impl(np.asarray(query), np.asarray(docs), N_CORES)
